# revision 16
# baseline (speedup 1.0000x reference)
"""Trainium2 Bass kernel for LocallyDirected1D — TensorE edition.

out[b, j] = sum_{e in segment j} x[b, e] * k[e]  (+ bias[j]);
mask_col sorted => segments are contiguous runs of the edge list.

Design:
  * Edge-sharding: core c handles a contiguous column range holding
    ~E/8 edges (split at segment boundaries), with ALL 64 batch rows.
  * Edges grouped in chunks of K=128 (partition/contraction dim).  Per
    chunk the host builds S [128, 8] bf16 with S[p, m] = k[e_p] iff
    col(e_p) == firstcol(chunk) + m  (a 128-edge chunk spans <= 7
    segments since the minimum segment length is 25).
  * Device: ONE matmul per chunk: psum[64 b, 8 segs] = xchunk.T @ S,
    with lhsT (stationary) = x chunk [128 edges, 64 batch] bf16.
  * Chunks PAIR into psum row halves: even chunk -> rows 0:64
    (PE col-strips 0-1), odd -> rows 64:128 (strips 2-3).  The
    col-tiled weight loads of one half overlap the other half's
    matmuls: measured ~29 ns/chunk vs ~54 unpaired.  The has_written
    clear of start=True is per-col-tile, so the first chunk of EACH
    half uses start=True (stale-psum protection on bank reuse).
  * 128 chunks share one PSUM bank [128, 512] (8 f32 per chunk slot).
  * ScalarE (own ports) evacuates banks to bf16 SBUF; per-window
    output DMAs ride the ACT HWDGE ring so they never head-of-line
    block the slab loads (sync ring).  Host scatter-adds the
    per-chunk partials (boundary segments span adjacent chunks).
  * The program is identical across cores (SPMD): all data dependence
    lives in the host-packed slabs; slabs are flattened window-major
    so every DMA reads a contiguous flat range.
"""

import numpy as np
from ml_dtypes import bfloat16

import concourse.bass as bass
import concourse.mybir as mybir
from concourse.tile import TileContext
from concourse.bass_utils import run_bass_kernel_spmd

B = 64
E = 1_000_000
NOUT = 20_000
NCORES = 8
CHK = 128                   # edges per chunk (contraction dim)
MFIX = 8                    # psum columns per chunk (max segment span)
NCW = 128                   # max chunks per psum window (2 halves x 64)
EPC = E // NCORES           # edges per core (target)
NCH = (EPC + CHK - 1) // CHK + 1   # chunks per core (uniform, padded)


def _window_sizes():
    """Ramped window sizes: small first windows for fast pipeline start,
    small last windows for a short drain."""
    head = [16, 24, 32, 48, 64, 96]
    tail = [32, 16]
    left = NCH - sum(head) - sum(tail)
    mid = []
    while left > NCW:
        mid.append(NCW)
        left -= NCW
    if left:
        mid.append(left)
    return head + mid + tail


WSIZES = _window_sizes()
NW = len(WSIZES)
WCHUNK0 = np.concatenate([[0], np.cumsum(WSIZES)]).astype(int)
WCOLS = [((n + 1) // 2) * MFIX for n in WSIZES]    # psum/evac cols per win
WOBASE = np.concatenate([[0], np.cumsum(WCOLS)]).astype(int)
OBW = int(WOBASE[-1])              # obuf columns (128 rows)
ROWW = B + 2                       # slab width per chunk: x | k | segidx

F32 = mybir.dt.float32
BF16 = mybir.dt.bfloat16


def _build_program():
    nc = bass.Bass()
    xs_d = nc.dram_tensor("xsl", [128 * NCH * ROWW], BF16,
                          kind="ExternalInput")
    i_d = nc.dram_tensor("iota8", [128 * MFIX], BF16, kind="ExternalInput")
    o_d = nc.dram_tensor("obuf", [128 * OBW], BF16, kind="ExternalOutput")

    with TileContext(nc) as tc:
        with (
            tc.tile_pool(name="xp", bufs=6) as xp,
            tc.tile_pool(name="sp", bufs=4) as sp,
            tc.psum_pool(name="pp", bufs=4) as pp,
            tc.tile_pool(name="op", bufs=1) as op_,
        ):
            ob = op_.tile([128, OBW], BF16, tag="ob")
            it = op_.tile([128, MFIX], BF16, tag="iota")
            # the iota constant is 128 tiny (16 B) descriptors, each paying
            # the ~800 ns minimum HBM round-trip: on the sync ring's FIFO it
            # head-of-line blocks the first slab for ~6 us.  Load it on the
            # ACT ring instead, clear of all slab loads.
            nc.scalar.dma_start(
                it[:], i_d[:].rearrange("(j f) -> j f", j=128))
            for w in range(NW):
                ncw = WSIZES[w]
                c0 = int(WCHUNK0[w])
                xt = xp.tile([128, ncw * ROWW], BF16, tag="x")
                nc.sync.dma_start(
                    xt[:],
                    xs_d[128 * c0 * ROWW:128 * (c0 + ncw) * ROWW]
                    .rearrange("(j f) -> j f", j=128))
                XF = ncw * B
                # build S [128, ncw, 8] on the otherwise idle VectorE:
                # S[p, l, m] = (segidx[p, l] == m) * k[p, l]
                st = sp.tile([128, ncw * MFIX], BF16, tag="s")
                sv = st[:].rearrange("j (l m) -> j l m", m=MFIX)
                kv = (xt[:, XF:XF + ncw].unsqueeze(2)
                      .broadcast_to([128, ncw, MFIX]))
                iv = (xt[:, XF + ncw:XF + 2 * ncw].unsqueeze(2)
                      .broadcast_to([128, ncw, MFIX]))
                i8 = (it[:].unsqueeze(1)
                      .broadcast_to([128, ncw, MFIX]))
                nc.vector.tensor_tensor(sv, iv, i8,
                                        mybir.AluOpType.is_equal)
                nc.vector.tensor_tensor(sv, sv, kv,
                                        mybir.AluOpType.mult)
                ps = pp.tile([128, 512], F32, tag="ps")
                for l in range(ncw):
                    r0 = 64 * (l % 2)
                    cc = (l // 2) * MFIX
                    nc.tensor.matmul(
                        ps[r0:r0 + 64, cc:cc + MFIX],
                        xt[:, l * B:(l + 1) * B],
                        st[:, l * MFIX:(l + 1) * MFIX],
                        start=(l <= 1), stop=(l == ncw - 1),
                    )
                wbase = int(WOBASE[w])
                wcols = WCOLS[w]
                nc.scalar.copy(ob[:, wbase:wbase + wcols],
                               ps[:, 0:wcols])
                # output store on the ACT HWDGE ring: never head-of-line
                # blocks the slab loads (sync ring)
                nc.scalar.dma_start(
                    o_d[:].rearrange("(j f) -> j f", j=128)
                    [:, wbase:wbase + wcols],
                    ob[:, wbase:wbase + wcols])
    return nc


def _split_multi_waits(nc):
    """walrus allows at most one sync-wait per engine instruction; hoist
    extra waits into standalone EventSemaphore sequencer instructions."""
    from bass_rust import SyncInfo
    n = 0
    for f in nc.m.functions:
        for blk in f.blocks:
            new = []
            for inst in blk.instructions:
                si = inst.sync_info
                if si is not None and len(si.on_wait) > 1:
                    for wt in si.on_wait[:-1]:
                        n += 1
                        new.append(mybir.InstEventSemaphore(
                            name=f"evw-{n}", engine=inst.engine,
                            sync_info=SyncInfo(on_wait=[wt], on_update=[]),
                        ))
                    inst.sync_info = SyncInfo(on_wait=[si.on_wait[-1]],
                                              on_update=list(si.on_update))
                new.append(inst)
            try:
                blk.instructions = new
            except Exception:
                blk.instructions[:] = new
    return n


def _plan(mask_col):
    """Per-core column boundaries with ~equal edge counts."""
    o = np.searchsorted(mask_col, np.arange(NOUT + 1)).astype(np.int64)
    targets = (np.arange(NCORES + 1) * E) // NCORES
    cb = np.searchsorted(o, targets)
    cb[0], cb[NCORES] = 0, NOUT
    return o, cb


def kernel(x, kernel, bias, mask_row, mask_col, _trace=False):
    x = np.asarray(x, np.float32)
    kflat = np.asarray(kernel, np.float32).reshape(E)
    bias = np.asarray(bias, np.float32)
    mask_col = np.asarray(mask_col)
    x2 = np.ascontiguousarray(x.reshape(B, E))
    cols = mask_col.astype(np.int64)

    o, cb = _plan(cols)
    nc = _build_program()
    _split_multi_waits(nc)

    xb = x2.astype(bfloat16)
    kb = kflat.astype(bfloat16)

    in_maps = []
    chunk_first = []
    for c in range(NCORES):
        e0, e1 = int(o[cb[c]]), int(o[cb[c + 1]])
        ne = e1 - e0
        nch_used = (ne + CHK - 1) // CHK
        assert nch_used <= NCH, (ne, NCH)
        xsl = np.zeros((128, NCH * B), bfloat16)
        ksl = np.zeros((128, NCH), bfloat16)
        isl = np.zeros((128, NCH), bfloat16)
        firsts = np.zeros(NCH, np.int64)
        for ci in range(nch_used):
            s = e0 + ci * CHK
            n = min(CHK, e1 - s)
            cc = cols[s:s + n]
            f0 = int(cc[0])
            firsts[ci] = f0
            assert int(cc[-1]) - f0 < MFIX
            xsl[:n, ci * B:(ci + 1) * B] = xb[:, s:s + n].T
            ksl[:n, ci] = kb[s:s + n]
            isl[:n, ci] = (cc - f0).astype(bfloat16)
            isl[n:, ci] = MFIX       # padded rows match no iota column
        if nch_used < NCH:
            isl[:, nch_used:] = MFIX
        chunk_first.append(firsts)
        # flatten window-major with x | k | segidx per window: device
        # DMAs slice contiguous flat ranges of the single packed slab
        xw = []
        for w in range(NW):
            a, b = int(WCHUNK0[w]), int(WCHUNK0[w + 1])
            xw.append(np.concatenate(
                [xsl[:, a * B:b * B], ksl[:, a:b], isl[:, a:b]],
                axis=1).ravel())
        in_maps.append({"xsl": np.concatenate(xw),
                        "iota8": np.tile(
                            np.arange(MFIX, dtype=np.float32), 128)
                        .astype(bfloat16)})

    res = run_bass_kernel_spmd(
        nc, in_maps, core_ids=list(range(NCORES)), trace=_trace)

    out_full = np.zeros((B, NOUT + MFIX), np.float32)
    for c in range(NCORES):
        ob = (np.asarray(res.results[c]["obuf"]).astype(np.float32)
              .reshape(128, OBW))
        vals = np.zeros((NCH, MFIX, B), np.float32)
        for w in range(NW):
            ncw = WSIZES[w]
            wc = ob[:, int(WOBASE[w]):int(WOBASE[w]) + WCOLS[w]]
            wc = wc.reshape(2, 64, -1, MFIX)     # [half, b, slot, m]
            for half in range(2):
                idx = np.arange(half, ncw, 2)
                vals[int(WCHUNK0[w]) + idx] = (
                    wc[half, :, :len(idx)].transpose(1, 2, 0))
        firsts = chunk_first[c]
        segidx = (firsts[:, None] + np.arange(MFIX)[None, :]).reshape(-1)
        np.add.at(out_full.transpose(1, 0), segidx,
                  vals.reshape(NCH * MFIX, B))
    out = out_full[:, :NOUT, None] + bias[None, :, :]
    if _trace:
        return out, res
    return out


# revision 21
# speedup vs baseline: 1.0355x; 1.0355x over previous
"""Trainium2 Bass kernel for LocallyDirected1D — TensorE edition.

out[b, j] = sum_{e in segment j} x[b, e] * k[e]  (+ bias[j]);
mask_col sorted => segments are contiguous runs of the edge list.

Design:
  * Edge-sharding: core c handles a contiguous column range holding
    ~E/8 edges (split at segment boundaries), with ALL 64 batch rows.
  * Edges grouped in chunks of K=128 (partition/contraction dim).  Per
    chunk the host builds S [128, 8] bf16 with S[p, m] = k[e_p] iff
    col(e_p) == firstcol(chunk) + m  (a 128-edge chunk spans <= 7
    segments since the minimum segment length is 25).
  * Device: ONE matmul per chunk: psum[64 b, 8 segs] = xchunk.T @ S,
    with lhsT (stationary) = x chunk [128 edges, 64 batch] bf16.
  * Chunks PAIR into psum row halves: even chunk -> rows 0:64
    (PE col-strips 0-1), odd -> rows 64:128 (strips 2-3).  The
    col-tiled weight loads of one half overlap the other half's
    matmuls: measured ~29 ns/chunk vs ~54 unpaired.  The has_written
    clear of start=True is per-col-tile, so the first chunk of EACH
    half uses start=True (stale-psum protection on bank reuse).
  * 128 chunks share one PSUM bank [128, 512] (8 f32 per chunk slot).
  * ScalarE (own ports) evacuates banks to bf16 SBUF; per-window
    output DMAs ride the ACT HWDGE ring so they never head-of-line
    block the slab loads (sync ring).  Host scatter-adds the
    per-chunk partials (boundary segments span adjacent chunks).
  * The program is identical across cores (SPMD): all data dependence
    lives in the host-packed slabs; slabs are flattened window-major
    so every DMA reads a contiguous flat range.
"""

import numpy as np
from ml_dtypes import bfloat16

import concourse.bass as bass
import concourse.mybir as mybir
from concourse.tile import TileContext
from concourse.bass_utils import run_bass_kernel_spmd

B = 64
E = 1_000_000
NOUT = 20_000
NCORES = 8
CHK = 128                   # edges per chunk (contraction dim)
MFIX = 8                    # psum columns per chunk (max segment span)
NCW = 128                   # max chunks per psum window (2 halves x 64)
EPC = E // NCORES           # edges per core (target)
NCH = (EPC + CHK - 1) // CHK + 1   # chunks per core (uniform, padded)


def _window_sizes():
    """Ramped window sizes: small first windows for fast pipeline start,
    big mid windows (4.3 MB DMAs amortize per-transfer fixed cost; each
    splits into two 128-chunk psum sub-banks), small last for the drain."""
    head = [16, 24, 32, 48, 64, 96]
    tail = [32, 16]
    left = NCH - sum(head) - sum(tail)
    mid = []
    while left > 2 * NCW:
        mid.append(2 * NCW)
        left -= 2 * NCW
    if left:
        mid.append(left)
    return head + mid + tail


WSIZES = _window_sizes()
NW = len(WSIZES)
WCHUNK0 = np.concatenate([[0], np.cumsum(WSIZES)]).astype(int)
# per window: psum sub-banks of <=NCW chunks, each ceil(s/2)*MFIX cols
WSUBS = [[min(NCW, n - i) for i in range(0, n, NCW)] for n in WSIZES]
WCOLS = [sum(((s + 1) // 2) * MFIX for s in subs) for subs in WSUBS]
WOBASE = np.concatenate([[0], np.cumsum(WCOLS)]).astype(int)
OBW = int(WOBASE[-1])              # obuf columns (128 rows)
ROWW = B + 2                       # slab width per chunk: x | k | segidx

F32 = mybir.dt.float32
BF16 = mybir.dt.bfloat16


def _build_program():
    nc = bass.Bass()
    xs_d = nc.dram_tensor("xsl", [128 * NCH * ROWW], BF16,
                          kind="ExternalInput")
    i_d = nc.dram_tensor("iota8", [128 * MFIX], BF16, kind="ExternalInput")
    o_d = nc.dram_tensor("obuf", [128 * OBW], BF16, kind="ExternalOutput")

    with TileContext(nc) as tc:
        with (
            tc.tile_pool(name="xp", bufs=4) as xp,
            tc.tile_pool(name="sp", bufs=4) as sp,
            tc.psum_pool(name="pp", bufs=4) as pp,
            tc.tile_pool(name="op", bufs=1) as op_,
        ):
            ob = op_.tile([128, OBW], BF16, tag="ob")
            it = op_.tile([128, MFIX], BF16, tag="iota")
            nc.sync.dma_start(
                it[:], i_d[:].rearrange("(j f) -> j f", j=128))
            for w in range(NW):
                ncw = WSIZES[w]
                c0 = int(WCHUNK0[w])
                xt = xp.tile([128, ncw * ROWW], BF16, tag="x")
                nc.sync.dma_start(
                    xt[:],
                    xs_d[128 * c0 * ROWW:128 * (c0 + ncw) * ROWW]
                    .rearrange("(j f) -> j f", j=128))
                XF = ncw * B
                # build S [128, ncw, 8] on the otherwise idle VectorE:
                # S[p, l, m] = (segidx[p, l] == m) * k[p, l]
                st = sp.tile([128, ncw * MFIX], BF16, tag="s")
                sv = st[:].rearrange("j (l m) -> j l m", m=MFIX)
                kv = (xt[:, XF:XF + ncw].unsqueeze(2)
                      .broadcast_to([128, ncw, MFIX]))
                iv = (xt[:, XF + ncw:XF + 2 * ncw].unsqueeze(2)
                      .broadcast_to([128, ncw, MFIX]))
                i8 = (it[:].unsqueeze(1)
                      .broadcast_to([128, ncw, MFIX]))
                nc.vector.tensor_tensor(sv, iv, i8,
                                        mybir.AluOpType.is_equal)
                nc.vector.tensor_tensor(sv, sv, kv,
                                        mybir.AluOpType.mult)
                sbase = int(WOBASE[w])
                g0 = 0
                for scw in WSUBS[w]:
                    ps = pp.tile([128, 512], F32, tag="ps")
                    for l in range(scw):
                        g = g0 + l
                        r0 = 64 * (l % 2)
                        cc = (l // 2) * MFIX
                        nc.tensor.matmul(
                            ps[r0:r0 + 64, cc:cc + MFIX],
                            xt[:, g * B:(g + 1) * B],
                            st[:, g * MFIX:(g + 1) * MFIX],
                            start=(l <= 1), stop=(l == scw - 1),
                        )
                    scols = ((scw + 1) // 2) * MFIX
                    nc.scalar.copy(ob[:, sbase:sbase + scols],
                                   ps[:, 0:scols])
                    # output store on the ACT HWDGE ring: never head-of-
                    # line blocks the slab loads (sync ring)
                    nc.scalar.dma_start(
                        o_d[:].rearrange("(j f) -> j f", j=128)
                        [:, sbase:sbase + scols],
                        ob[:, sbase:sbase + scols])
                    sbase += scols
                    g0 += scw
    return nc


def _split_multi_waits(nc):
    """walrus allows at most one sync-wait per engine instruction; hoist
    extra waits into standalone EventSemaphore sequencer instructions."""
    from bass_rust import SyncInfo
    n = 0
    for f in nc.m.functions:
        for blk in f.blocks:
            new = []
            for inst in blk.instructions:
                si = inst.sync_info
                if si is not None and len(si.on_wait) > 1:
                    for wt in si.on_wait[:-1]:
                        n += 1
                        new.append(mybir.InstEventSemaphore(
                            name=f"evw-{n}", engine=inst.engine,
                            sync_info=SyncInfo(on_wait=[wt], on_update=[]),
                        ))
                    inst.sync_info = SyncInfo(on_wait=[si.on_wait[-1]],
                                              on_update=list(si.on_update))
                new.append(inst)
            try:
                blk.instructions = new
            except Exception:
                blk.instructions[:] = new
    return n


def _plan(mask_col):
    """Per-core column boundaries with ~equal edge counts."""
    o = np.searchsorted(mask_col, np.arange(NOUT + 1)).astype(np.int64)
    targets = (np.arange(NCORES + 1) * E) // NCORES
    cb = np.searchsorted(o, targets)
    cb[0], cb[NCORES] = 0, NOUT
    return o, cb


def kernel(x, kernel, bias, mask_row, mask_col, _trace=False):
    x = np.asarray(x, np.float32)
    kflat = np.asarray(kernel, np.float32).reshape(E)
    bias = np.asarray(bias, np.float32)
    mask_col = np.asarray(mask_col)
    x2 = np.ascontiguousarray(x.reshape(B, E))
    cols = mask_col.astype(np.int64)

    o, cb = _plan(cols)
    nc = _build_program()
    _split_multi_waits(nc)

    xb = x2.astype(bfloat16)
    kb = kflat.astype(bfloat16)

    in_maps = []
    chunk_first = []
    for c in range(NCORES):
        e0, e1 = int(o[cb[c]]), int(o[cb[c + 1]])
        ne = e1 - e0
        nch_used = (ne + CHK - 1) // CHK
        assert nch_used <= NCH, (ne, NCH)
        xsl = np.zeros((128, NCH * B), bfloat16)
        ksl = np.zeros((128, NCH), bfloat16)
        isl = np.zeros((128, NCH), bfloat16)
        firsts = np.zeros(NCH, np.int64)
        for ci in range(nch_used):
            s = e0 + ci * CHK
            n = min(CHK, e1 - s)
            cc = cols[s:s + n]
            f0 = int(cc[0])
            firsts[ci] = f0
            assert int(cc[-1]) - f0 < MFIX
            xsl[:n, ci * B:(ci + 1) * B] = xb[:, s:s + n].T
            ksl[:n, ci] = kb[s:s + n]
            isl[:n, ci] = (cc - f0).astype(bfloat16)
            isl[n:, ci] = MFIX       # padded rows match no iota column
        if nch_used < NCH:
            isl[:, nch_used:] = MFIX
        chunk_first.append(firsts)
        # flatten window-major with x | k | segidx per window: device
        # DMAs slice contiguous flat ranges of the single packed slab
        xw = []
        for w in range(NW):
            a, b = int(WCHUNK0[w]), int(WCHUNK0[w + 1])
            xw.append(np.concatenate(
                [xsl[:, a * B:b * B], ksl[:, a:b], isl[:, a:b]],
                axis=1).ravel())
        in_maps.append({"xsl": np.concatenate(xw),
                        "iota8": np.tile(
                            np.arange(MFIX, dtype=np.float32), 128)
                        .astype(bfloat16)})

    res = run_bass_kernel_spmd(
        nc, in_maps, core_ids=list(range(NCORES)), trace=_trace)

    out_full = np.zeros((B, NOUT + MFIX), np.float32)
    for c in range(NCORES):
        ob = (np.asarray(res.results[c]["obuf"]).astype(np.float32)
              .reshape(128, OBW))
        vals = np.zeros((NCH, MFIX, B), np.float32)
        for w in range(NW):
            sbase = int(WOBASE[w])
            g0 = int(WCHUNK0[w])
            for scw in WSUBS[w]:
                scols = ((scw + 1) // 2) * MFIX
                wc = ob[:, sbase:sbase + scols]
                wc = wc.reshape(2, 64, -1, MFIX)   # [half, b, slot, m]
                for half in range(2):
                    idx = np.arange(half, scw, 2)
                    vals[g0 + idx] = (
                        wc[half, :, :len(idx)].transpose(1, 2, 0))
                sbase += scols
                g0 += scw
        firsts = chunk_first[c]
        segidx = (firsts[:, None] + np.arange(MFIX)[None, :]).reshape(-1)
        np.add.at(out_full.transpose(1, 0), segidx,
                  vals.reshape(NCH * MFIX, B))
    out = out_full[:, :NOUT, None] + bias[None, :, :]
    if _trace:
        return out, res
    return out


# revision 25
# speedup vs baseline: 1.0420x; 1.0063x over previous
"""Trainium2 Bass kernel for LocallyDirected1D — TensorE edition.

out[b, j] = sum_{e in segment j} x[b, e] * k[e]  (+ bias[j]);
mask_col sorted => segments are contiguous runs of the edge list.

Design:
  * Edge-sharding: core c handles a contiguous column range holding
    ~E/8 edges (split at segment boundaries), with ALL 64 batch rows.
  * Edges grouped in chunks of K=128 (partition/contraction dim).  Per
    chunk the host builds S [128, 8] bf16 with S[p, m] = k[e_p] iff
    col(e_p) == firstcol(chunk) + m  (a 128-edge chunk spans <= 7
    segments since the minimum segment length is 25).
  * Device: ONE matmul per chunk: psum[64 b, 8 segs] = xchunk.T @ S,
    with lhsT (stationary) = x chunk [128 edges, 64 batch] bf16.
  * Chunks PAIR into psum row halves: even chunk -> rows 0:64
    (PE col-strips 0-1), odd -> rows 64:128 (strips 2-3).  The
    col-tiled weight loads of one half overlap the other half's
    matmuls: measured ~29 ns/chunk vs ~54 unpaired.  The has_written
    clear of start=True is per-col-tile, so the first chunk of EACH
    half uses start=True (stale-psum protection on bank reuse).
  * 128 chunks share one PSUM bank [128, 512] (8 f32 per chunk slot).
  * ScalarE (own ports) evacuates banks to bf16 SBUF; per-window
    output DMAs ride the ACT HWDGE ring so they never head-of-line
    block the slab loads (sync ring).  Host scatter-adds the
    per-chunk partials (boundary segments span adjacent chunks).
  * The program is identical across cores (SPMD): all data dependence
    lives in the host-packed slabs; slabs are flattened window-major
    so every DMA reads a contiguous flat range.
"""

import numpy as np
from ml_dtypes import bfloat16

import concourse.bass as bass
import concourse.mybir as mybir
from concourse.tile import TileContext
from concourse.bass_utils import run_bass_kernel_spmd

B = 64
E = 1_000_000
NOUT = 20_000
NCORES = 8
CHK = 128                   # edges per chunk (contraction dim)
MFIX = 8                    # psum columns per chunk (max segment span)
NCW = 128                   # max chunks per psum window (2 halves x 64)
EPC = E // NCORES           # edges per core (target)
NCH = (EPC + CHK - 1) // CHK + 1   # chunks per core (uniform, padded)


def _window_sizes():
    """Ramped window sizes: small first windows for fast pipeline start,
    small last windows for a short drain."""
    head = [16, 24, 32, 48, 64, 96]
    tail = [32, 16]
    left = NCH - sum(head) - sum(tail)
    mid = []
    while left > NCW:
        mid.append(NCW)
        left -= NCW
    if left:
        mid.append(left)
    return head + mid + tail


WSIZES = _window_sizes()
NW = len(WSIZES)
WCHUNK0 = np.concatenate([[0], np.cumsum(WSIZES)]).astype(int)
WCOLS = [((n + 1) // 2) * MFIX for n in WSIZES]    # psum/evac cols per win
WOBASE = np.concatenate([[0], np.cumsum(WCOLS)]).astype(int)
OBW = int(WOBASE[-1])              # obuf columns (128 rows)
ROWW = B + 2                       # slab width per chunk: x | k | segidx

F32 = mybir.dt.float32
BF16 = mybir.dt.bfloat16


def _build_program():
    nc = bass.Bass()
    xs_d = nc.dram_tensor("xsl", [128 * NCH * ROWW], BF16,
                          kind="ExternalInput")
    o_d = nc.dram_tensor("obuf", [128 * OBW], BF16, kind="ExternalOutput")

    with TileContext(nc) as tc:
        with (
            tc.tile_pool(name="xp", bufs=6) as xp,
            tc.tile_pool(name="sp", bufs=4) as sp,
            tc.psum_pool(name="pp", bufs=4) as pp,
            tc.tile_pool(name="op", bufs=1) as op_,
        ):
            ob = op_.tile([128, OBW], BF16, tag="ob")
            # build the 0..7 comparison row on-device (one-time): avoids
            # a 128-tiny-descriptor constant DMA at the sync ring's head
            it = op_.tile([128, MFIX], BF16, tag="iota")
            iti = op_.tile([128, MFIX], mybir.dt.int16, tag="iotai")
            nc.gpsimd.iota(iti[:], pattern=[[1, MFIX]], base=0,
                           channel_multiplier=0)
            nc.vector.tensor_copy(it[:], iti[:])
            for w in range(NW):
                ncw = WSIZES[w]
                c0 = int(WCHUNK0[w])
                xt = xp.tile([128, ncw * ROWW], BF16, tag="x")
                nc.sync.dma_start(
                    xt[:],
                    xs_d[128 * c0 * ROWW:128 * (c0 + ncw) * ROWW]
                    .rearrange("(j f) -> j f", j=128))
                XF = ncw * B
                # build S [128, ncw, 8] on the otherwise idle VectorE:
                # S[p, l, m] = (segidx[p, l] == m) * k[p, l]
                st = sp.tile([128, ncw * MFIX], BF16, tag="s")
                sv = st[:].rearrange("j (l m) -> j l m", m=MFIX)
                kv = (xt[:, XF:XF + ncw].unsqueeze(2)
                      .broadcast_to([128, ncw, MFIX]))
                iv = (xt[:, XF + ncw:XF + 2 * ncw].unsqueeze(2)
                      .broadcast_to([128, ncw, MFIX]))
                i8 = (it[:].unsqueeze(1)
                      .broadcast_to([128, ncw, MFIX]))
                nc.vector.tensor_tensor(sv, iv, i8,
                                        mybir.AluOpType.is_equal)
                nc.vector.tensor_tensor(sv, sv, kv,
                                        mybir.AluOpType.mult)
                ps = pp.tile([128, 512], F32, tag="ps")
                for l in range(ncw):
                    r0 = 64 * (l % 2)
                    cc = (l // 2) * MFIX
                    nc.tensor.matmul(
                        ps[r0:r0 + 64, cc:cc + MFIX],
                        xt[:, l * B:(l + 1) * B],
                        st[:, l * MFIX:(l + 1) * MFIX],
                        start=(l <= 1), stop=(l == ncw - 1),
                    )
                wbase = int(WOBASE[w])
                wcols = WCOLS[w]
                nc.scalar.copy(ob[:, wbase:wbase + wcols],
                               ps[:, 0:wcols])
                # output store on the ACT HWDGE ring: never head-of-line
                # blocks the slab loads (sync ring)
                nc.scalar.dma_start(
                    o_d[:].rearrange("(j f) -> j f", j=128)
                    [:, wbase:wbase + wcols],
                    ob[:, wbase:wbase + wcols])
    return nc


def _split_multi_waits(nc):
    """walrus allows at most one sync-wait per engine instruction; hoist
    extra waits into standalone EventSemaphore sequencer instructions."""
    from bass_rust import SyncInfo
    n = 0
    for f in nc.m.functions:
        for blk in f.blocks:
            new = []
            for inst in blk.instructions:
                si = inst.sync_info
                if si is not None and len(si.on_wait) > 1:
                    for wt in si.on_wait[:-1]:
                        n += 1
                        new.append(mybir.InstEventSemaphore(
                            name=f"evw-{n}", engine=inst.engine,
                            sync_info=SyncInfo(on_wait=[wt], on_update=[]),
                        ))
                    inst.sync_info = SyncInfo(on_wait=[si.on_wait[-1]],
                                              on_update=list(si.on_update))
                new.append(inst)
            try:
                blk.instructions = new
            except Exception:
                blk.instructions[:] = new
    return n


def _plan(mask_col):
    """Per-core column boundaries with ~equal edge counts."""
    o = np.searchsorted(mask_col, np.arange(NOUT + 1)).astype(np.int64)
    targets = (np.arange(NCORES + 1) * E) // NCORES
    cb = np.searchsorted(o, targets)
    cb[0], cb[NCORES] = 0, NOUT
    return o, cb


def kernel(x, kernel, bias, mask_row, mask_col, _trace=False):
    x = np.asarray(x, np.float32)
    kflat = np.asarray(kernel, np.float32).reshape(E)
    bias = np.asarray(bias, np.float32)
    mask_col = np.asarray(mask_col)
    x2 = np.ascontiguousarray(x.reshape(B, E))
    cols = mask_col.astype(np.int64)

    o, cb = _plan(cols)
    nc = _build_program()
    _split_multi_waits(nc)

    xb = x2.astype(bfloat16)
    kb = kflat.astype(bfloat16)

    in_maps = []
    chunk_first = []
    for c in range(NCORES):
        e0, e1 = int(o[cb[c]]), int(o[cb[c + 1]])
        ne = e1 - e0
        nch_used = (ne + CHK - 1) // CHK
        assert nch_used <= NCH, (ne, NCH)
        xsl = np.zeros((128, NCH * B), bfloat16)
        ksl = np.zeros((128, NCH), bfloat16)
        isl = np.zeros((128, NCH), bfloat16)
        firsts = np.zeros(NCH, np.int64)
        for ci in range(nch_used):
            s = e0 + ci * CHK
            n = min(CHK, e1 - s)
            cc = cols[s:s + n]
            f0 = int(cc[0])
            firsts[ci] = f0
            assert int(cc[-1]) - f0 < MFIX
            xsl[:n, ci * B:(ci + 1) * B] = xb[:, s:s + n].T
            ksl[:n, ci] = kb[s:s + n]
            isl[:n, ci] = (cc - f0).astype(bfloat16)
            isl[n:, ci] = MFIX       # padded rows match no iota column
        if nch_used < NCH:
            isl[:, nch_used:] = MFIX
        chunk_first.append(firsts)
        # flatten window-major with x | k | segidx per window: device
        # DMAs slice contiguous flat ranges of the single packed slab
        xw = []
        for w in range(NW):
            a, b = int(WCHUNK0[w]), int(WCHUNK0[w + 1])
            xw.append(np.concatenate(
                [xsl[:, a * B:b * B], ksl[:, a:b], isl[:, a:b]],
                axis=1).ravel())
        in_maps.append({"xsl": np.concatenate(xw)})

    res = run_bass_kernel_spmd(
        nc, in_maps, core_ids=list(range(NCORES)), trace=_trace)

    out_full = np.zeros((B, NOUT + MFIX), np.float32)
    for c in range(NCORES):
        ob = (np.asarray(res.results[c]["obuf"]).astype(np.float32)
              .reshape(128, OBW))
        vals = np.zeros((NCH, MFIX, B), np.float32)
        for w in range(NW):
            ncw = WSIZES[w]
            wc = ob[:, int(WOBASE[w]):int(WOBASE[w]) + WCOLS[w]]
            wc = wc.reshape(2, 64, -1, MFIX)     # [half, b, slot, m]
            for half in range(2):
                idx = np.arange(half, ncw, 2)
                vals[int(WCHUNK0[w]) + idx] = (
                    wc[half, :, :len(idx)].transpose(1, 2, 0))
        firsts = chunk_first[c]
        segidx = (firsts[:, None] + np.arange(MFIX)[None, :]).reshape(-1)
        np.add.at(out_full.transpose(1, 0), segidx,
                  vals.reshape(NCH * MFIX, B))
    out = out_full[:, :NOUT, None] + bias[None, :, :]
    if _trace:
        return out, res
    return out


# revision 26
# speedup vs baseline: 1.1090x; 1.0643x over previous
"""Trainium2 Bass kernel for LocallyDirected1D — TensorE edition.

out[b, j] = sum_{e in segment j} x[b, e] * k[e]  (+ bias[j]);
mask_col sorted => segments are contiguous runs of the edge list.

Design:
  * Edge-sharding: core c handles a contiguous column range holding
    ~E/8 edges (split at segment boundaries), with ALL 64 batch rows.
  * Edges grouped in chunks of K=128 (partition/contraction dim).  Per
    chunk the host builds S [128, 8] bf16 with S[p, m] = k[e_p] iff
    col(e_p) == firstcol(chunk) + m  (a 128-edge chunk spans <= 7
    segments since the minimum segment length is 25).
  * Device: ONE matmul per chunk: psum[64 b, 8 segs] = xchunk.T @ S,
    with lhsT (stationary) = x chunk [128 edges, 64 batch] bf16.
  * Chunks PAIR into psum row halves: even chunk -> rows 0:64
    (PE col-strips 0-1), odd -> rows 64:128 (strips 2-3).  The
    col-tiled weight loads of one half overlap the other half's
    matmuls: measured ~29 ns/chunk vs ~54 unpaired.  The has_written
    clear of start=True is per-col-tile, so the first chunk of EACH
    half uses start=True (stale-psum protection on bank reuse).
  * 128 chunks share one PSUM bank [128, 512] (8 f32 per chunk slot).
  * ScalarE (own ports) evacuates banks to bf16 SBUF; per-window
    output DMAs ride the ACT HWDGE ring so they never head-of-line
    block the slab loads (sync ring).  Host scatter-adds the
    per-chunk partials (boundary segments span adjacent chunks).
  * The program is identical across cores (SPMD): all data dependence
    lives in the host-packed slabs; slabs are flattened window-major
    so every DMA reads a contiguous flat range.
"""

import numpy as np
from ml_dtypes import bfloat16

import concourse.bass as bass
import concourse.mybir as mybir
from concourse.tile import TileContext
from concourse.bass_utils import run_bass_kernel_spmd

B = 64
E = 1_000_000
NOUT = 20_000
NCORES = 8
CHK = 128                   # edges per chunk (contraction dim)
MFIX = 8                    # psum columns per chunk (max segment span)
NCW = 128                   # max chunks per psum window (2 halves x 64)
EPC = E // NCORES           # edges per core (target)
NCH = (EPC + CHK - 1) // CHK + 1   # chunks per core (uniform, padded)


def _window_sizes():
    """Ramped window sizes: small first windows for fast pipeline start,
    small last windows for a short drain."""
    head = [16, 24, 32, 48, 64, 96]
    tail = [32, 16]
    left = NCH - sum(head) - sum(tail)
    mid = []
    while left > NCW:
        mid.append(NCW)
        left -= NCW
    if left:
        mid.append(left)
    return head + mid + tail


WSIZES = _window_sizes()
NW = len(WSIZES)
WCHUNK0 = np.concatenate([[0], np.cumsum(WSIZES)]).astype(int)
WCOLS = [((n + 1) // 2) * MFIX for n in WSIZES]    # psum/evac cols per win
WOBASE = np.concatenate([[0], np.cumsum(WCOLS)]).astype(int)
OBW = int(WOBASE[-1])              # obuf columns (128 rows)
ROWW = B + 2                       # slab width per chunk: x | k | segidx

F32 = mybir.dt.float32
BF16 = mybir.dt.bfloat16


def _build_program():
    nc = bass.Bass()
    xs_d = nc.dram_tensor("xsl", [128 * NCH * ROWW], BF16,
                          kind="ExternalInput")
    i_d = nc.dram_tensor("iota8", [128 * MFIX], BF16, kind="ExternalInput")
    o_d = nc.dram_tensor("obuf", [128 * OBW], BF16, kind="ExternalOutput")

    with TileContext(nc) as tc:
        with (
            tc.tile_pool(name="xp", bufs=6) as xp,
            tc.tile_pool(name="sp", bufs=4) as sp,
            tc.psum_pool(name="pp", bufs=4) as pp,
            tc.tile_pool(name="op", bufs=1) as op_,
        ):
            ob = op_.tile([128, OBW], BF16, tag="ob")
            it = op_.tile([128, MFIX], BF16, tag="iota")
            nc.sync.dma_start(
                it[:], i_d[:].rearrange("(j f) -> j f", j=128))
            for w in range(NW):
                ncw = WSIZES[w]
                c0 = int(WCHUNK0[w])
                xt = xp.tile([128, ncw * ROWW], BF16, tag="x")
                nc.sync.dma_start(
                    xt[:],
                    xs_d[128 * c0 * ROWW:128 * (c0 + ncw) * ROWW]
                    .rearrange("(j f) -> j f", j=128))
                XF = ncw * B
                # build S [128, ncw, 8] on the otherwise idle VectorE:
                # S[p, l, m] = (segidx[p, l] == m) * k[p, l]
                st = sp.tile([128, ncw * MFIX], BF16, tag="s")
                sv = st[:].rearrange("j (l m) -> j l m", m=MFIX)
                kv = (xt[:, XF:XF + ncw].unsqueeze(2)
                      .broadcast_to([128, ncw, MFIX]))
                iv = (xt[:, XF + ncw:XF + 2 * ncw].unsqueeze(2)
                      .broadcast_to([128, ncw, MFIX]))
                i8 = (it[:].unsqueeze(1)
                      .broadcast_to([128, ncw, MFIX]))
                nc.vector.tensor_tensor(sv, iv, i8,
                                        mybir.AluOpType.is_equal)
                nc.vector.tensor_tensor(sv, sv, kv,
                                        mybir.AluOpType.mult)
                ps = pp.tile([128, 512], F32, tag="ps")
                for l in range(ncw):
                    r0 = 64 * (l % 2)
                    cc = (l // 2) * MFIX
                    nc.tensor.matmul(
                        ps[r0:r0 + 64, cc:cc + MFIX],
                        xt[:, l * B:(l + 1) * B],
                        st[:, l * MFIX:(l + 1) * MFIX],
                        start=(l <= 1), stop=(l == ncw - 1),
                    )
                wbase = int(WOBASE[w])
                wcols = WCOLS[w]
                nc.scalar.copy(ob[:, wbase:wbase + wcols],
                               ps[:, 0:wcols])
                # output store on the ACT HWDGE ring: never head-of-line
                # blocks the slab loads (sync ring)
                nc.scalar.dma_start(
                    o_d[:].rearrange("(j f) -> j f", j=128)
                    [:, wbase:wbase + wcols],
                    ob[:, wbase:wbase + wcols])
    return nc


def _split_multi_waits(nc):
    """walrus allows at most one sync-wait per engine instruction; hoist
    extra waits into standalone EventSemaphore sequencer instructions."""
    from bass_rust import SyncInfo
    n = 0
    for f in nc.m.functions:
        for blk in f.blocks:
            new = []
            for inst in blk.instructions:
                si = inst.sync_info
                if si is not None and len(si.on_wait) > 1:
                    for wt in si.on_wait[:-1]:
                        n += 1
                        new.append(mybir.InstEventSemaphore(
                            name=f"evw-{n}", engine=inst.engine,
                            sync_info=SyncInfo(on_wait=[wt], on_update=[]),
                        ))
                    inst.sync_info = SyncInfo(on_wait=[si.on_wait[-1]],
                                              on_update=list(si.on_update))
                new.append(inst)
            try:
                blk.instructions = new
            except Exception:
                blk.instructions[:] = new
    return n


def _plan(mask_col):
    """Per-core column boundaries with ~equal edge counts."""
    o = np.searchsorted(mask_col, np.arange(NOUT + 1)).astype(np.int64)
    targets = (np.arange(NCORES + 1) * E) // NCORES
    cb = np.searchsorted(o, targets)
    cb[0], cb[NCORES] = 0, NOUT
    return o, cb


def kernel(x, kernel, bias, mask_row, mask_col, _trace=False):
    x = np.asarray(x, np.float32)
    kflat = np.asarray(kernel, np.float32).reshape(E)
    bias = np.asarray(bias, np.float32)
    mask_col = np.asarray(mask_col)
    x2 = np.ascontiguousarray(x.reshape(B, E))
    cols = mask_col.astype(np.int64)

    o, cb = _plan(cols)
    nc = _build_program()
    _split_multi_waits(nc)

    xb = x2.astype(bfloat16)
    kb = kflat.astype(bfloat16)

    in_maps = []
    chunk_first = []
    for c in range(NCORES):
        e0, e1 = int(o[cb[c]]), int(o[cb[c + 1]])
        ne = e1 - e0
        nch_used = (ne + CHK - 1) // CHK
        assert nch_used <= NCH, (ne, NCH)
        xsl = np.zeros((128, NCH * B), bfloat16)
        ksl = np.zeros((128, NCH), bfloat16)
        isl = np.zeros((128, NCH), bfloat16)
        firsts = np.zeros(NCH, np.int64)
        for ci in range(nch_used):
            s = e0 + ci * CHK
            n = min(CHK, e1 - s)
            cc = cols[s:s + n]
            f0 = int(cc[0])
            firsts[ci] = f0
            assert int(cc[-1]) - f0 < MFIX
            xsl[:n, ci * B:(ci + 1) * B] = xb[:, s:s + n].T
            ksl[:n, ci] = kb[s:s + n]
            isl[:n, ci] = (cc - f0).astype(bfloat16)
            isl[n:, ci] = MFIX       # padded rows match no iota column
        if nch_used < NCH:
            isl[:, nch_used:] = MFIX
        chunk_first.append(firsts)
        # flatten window-major with x | k | segidx per window: device
        # DMAs slice contiguous flat ranges of the single packed slab
        xw = []
        for w in range(NW):
            a, b = int(WCHUNK0[w]), int(WCHUNK0[w + 1])
            xw.append(np.concatenate(
                [xsl[:, a * B:b * B], ksl[:, a:b], isl[:, a:b]],
                axis=1).ravel())
        in_maps.append({"xsl": np.concatenate(xw),
                        "iota8": np.tile(
                            np.arange(MFIX, dtype=np.float32), 128)
                        .astype(bfloat16)})

    res = run_bass_kernel_spmd(
        nc, in_maps, core_ids=list(range(NCORES)), trace=_trace)

    out_full = np.zeros((B, NOUT + MFIX), np.float32)
    for c in range(NCORES):
        ob = (np.asarray(res.results[c]["obuf"]).astype(np.float32)
              .reshape(128, OBW))
        vals = np.zeros((NCH, MFIX, B), np.float32)
        for w in range(NW):
            ncw = WSIZES[w]
            wc = ob[:, int(WOBASE[w]):int(WOBASE[w]) + WCOLS[w]]
            wc = wc.reshape(2, 64, -1, MFIX)     # [half, b, slot, m]
            for half in range(2):
                idx = np.arange(half, ncw, 2)
                vals[int(WCHUNK0[w]) + idx] = (
                    wc[half, :, :len(idx)].transpose(1, 2, 0))
        firsts = chunk_first[c]
        segidx = (firsts[:, None] + np.arange(MFIX)[None, :]).reshape(-1)
        np.add.at(out_full.transpose(1, 0), segidx,
                  vals.reshape(NCH * MFIX, B))
    out = out_full[:, :NOUT, None] + bias[None, :, :]
    if _trace:
        return out, res
    return out
